# revision 6
# baseline (speedup 1.0000x reference)
"""DenseGATConv Trainium2 kernel (8 NeuronCores).

Math (per batch b, head h, with e = exp o leaky_relu_{0.2}):
    feat = x @ W                          # [N, H*C]
    a_src[i,h] = <feat[i, h,:], att_src[h]> ; a_dst[j,h] likewise
    T[i,j,h]   = a_src[i,h] + a_dst[j,h]
    alpha      = adj_sl[i,j] * e(T)       # adj_sl = adj with unit diagonal
    D[j,h]     = sum_i alpha[i,j,h]       # normalization over axis i
    o[i,h,:]   = sum_j alpha[i,j,h] * feat[j,h,:] / D[j,h]  (+ bias)

Sharding: 8 cores = (b in 0..3) x (j-half in 0..1). Each core computes the
partial o-contribution of its 1024 destination columns j; the two halves of
each batch are summed on the host.

On-device layout is [j partitions, i free] (adj pre-transposed on the host).
Two interchangeable per-(h, j-tile) lattice pipelines produce
Z[j,i] (the matmul stream) and a per-j accumulator Dacc:

  ACT pipe (adj tile sent as BIG*(adj-1), additive mask):
     U = a_src_rep_h + adjm        (DVE tensor_tensor, bf16 2x)
     L = Prelu(U + a_dst[j]; 0.2)  (ScalarE)
     Z = Exp(L), Dacc = row-sums   (ScalarE, accum_out)   # Z = alpha
  DVE pipe (adj tile sent as 0/1, multiplicative mask), using
  e(T) = E2[j]*E1[i]*max(1, G'[i]*r'[j]) with
  E1=exp(a_src), G'=exp(-.8 a_src), E2=exp(a_dst), r'=exp(-.8 a_dst):
     K2 = max(G'_rep_h * r'[j], 1)          (tensor_scalar, bf16 4x)
     Y  = K2 * E1_rep_h                     (tensor_tensor, bf16 2x)
     Z  = Y * adj01                         (tensor_tensor)  # Z = alpha/E2[j]
     Dacc = row-sums via tensor_scalar copy w/ accum_out (bf16 4x)
  (for DVE-pipe tiles the missing E2[j] factor is folded into the
   per-tile matmul weights and into D = E2*Dacc.)

Output matmul per h (TensorE, bf16): lhsT = w[j,c] = (feat/D)[j,c] (times E2
for DVE-pipe tiles), rhs = Z[j,i] accumulated over j-tiles into PSUM [C, N].
"""

import os
import numpy as np
import ml_dtypes

from contextlib import ExitStack

import concourse.bass as bass
import concourse.tile as tile
from concourse import bacc, mybir
from concourse._compat import with_exitstack
from concourse.bass_utils import run_bass_kernel_spmd

F32 = mybir.dt.float32
BF16 = mybir.dt.bfloat16
AF = mybir.ActivationFunctionType
OP = mybir.AluOpType

B, N, F_IN, H, C = 4, 2048, 128, 4, 32
HC = H * C
J = N // 2          # destination columns per core
NT = J // 128       # j-tiles per core (8)
NEG_SLOPE = 0.2
BIG = 128.0         # additive mask magnitude for the ACT pipe
N_CORES = 8

# --- tuning knobs -----------------------------------------------------------
NT_ACT = 5          # j-tiles 0..NT_ACT-1 use the ACT pipe; rest the DVE pipe
GPS_Y = True        # run the DVE-pipe Y=K2*E1 multiply on GpSimd


@with_exitstack
def _gat_kernel(ctx: ExitStack, tc: tile.TileContext, outs, ins):
    nc = tc.nc
    adjm, xT, xTj, Wt, attb = ins      # dram APs
    opart = outs[0]                    # dram [H, C, N] f32

    pool = ctx.enter_context(tc.tile_pool(name="work", bufs=1))
    zpool = ctx.enter_context(tc.tile_pool(name="z", bufs=10))
    upool = ctx.enter_context(tc.tile_pool(name="u", bufs=2))
    lpool = ctx.enter_context(tc.tile_pool(name="l", bufs=2))
    rpool = ctx.enter_context(tc.tile_pool(name="reps", bufs=2))
    spool = ctx.enter_context(tc.tile_pool(name="small", bufs=1))
    dpool = ctx.enter_context(tc.tile_pool(name="dcols", bufs=1))
    opool = ctx.enter_context(tc.tile_pool(name="oevict", bufs=1))
    psum = ctx.enter_context(tc.tile_pool(name="ps", bufs=2, space="PSUM"))

    # ---- load inputs ----
    adj_sb = []                        # [128, 2048] bf16 per j-tile
    for t in range(NT):
        a = pool.tile([128, N], BF16, tag=f"adjm{t}")
        nc.sync.dma_start(a[:], adjm[t * 128:(t + 1) * 128, :])
        adj_sb.append(a)
    xT_sb = pool.tile([128, N], F32, tag="xT")
    nc.sync.dma_start(xT_sb[:], xT[:, :])
    xTj_sb = pool.tile([128, J], F32, tag="xTj")
    nc.sync.dma_start(xTj_sb[:], xTj[:, :])
    W_sb = spool.tile([128, HC], F32, tag="W")
    nc.sync.dma_start(W_sb[:], Wt[:, :])
    attb_sb = spool.tile([128, 8], F32, tag="attb")
    nc.sync.dma_start(attb_sb[:], attb[:, :])

    # ---- projections ----
    # featT[hc, n] = W.T @ xT  (full i range; used for i-side stats)
    featT_ps = psum.tile([128, N], F32, tag="ps")
    for q in range(N // 512):
        nc.tensor.matmul(featT_ps[:, q * 512:(q + 1) * 512], W_sb[:],
                         xT_sb[:, q * 512:(q + 1) * 512], start=True, stop=True)
    featT_sb = pool.tile([128, N], F32, tag="featT")
    nc.vector.tensor_copy(featT_sb[:], featT_ps[:])

    # feat_j[j, hc] per j-tile (for the matmul weights)
    fj_sb = []
    for t in range(NT):
        fj_ps = psum.tile([128, HC], F32, tag="ps")
        nc.tensor.matmul(fj_ps[:], xTj_sb[:, t * 128:(t + 1) * 128], W_sb[:],
                         start=True, stop=True)
        f = pool.tile([128, HC], F32, tag=f"fj{t}")
        nc.vector.tensor_copy(f[:], fj_ps[:])
        fj_sb.append(f)

    # ---- attention stats ----
    # i-side: a_srcT [4, N] = attb[:, :4].T @ featT
    stats_ps = psum.tile([8, N], F32, tag="ps")
    for q in range(N // 512):
        nc.tensor.matmul(stats_ps[:, q * 512:(q + 1) * 512], attb_sb[:],
                         featT_sb[:, q * 512:(q + 1) * 512], start=True, stop=True)
    asrc4 = spool.tile([4, N], F32, tag="asrc4")
    nc.vector.tensor_copy(asrc4[:], stats_ps[0:4, :])
    # bf16 rows to broadcast: a_src, G' = exp(-.8 a_src), E1 = exp(a_src)
    asrc_bf = spool.tile([4, N], BF16, tag="asrc_bf")
    nc.vector.tensor_copy(asrc_bf[:], asrc4[:])
    G1T = spool.tile([4, N], BF16, tag="G1T")
    nc.scalar.activation(G1T[:], asrc4[:], AF.Exp, scale=-(1.0 - NEG_SLOPE))
    E1T = spool.tile([4, N], BF16, tag="E1T")
    nc.scalar.activation(E1T[:], asrc4[:], AF.Exp)

    # j-side stats in [j partitions, stat] layout: first featTj [hc, j],
    # then per tile stats_j[j, s] via lhsT = featTj-chunk (contract hc)
    featTj_ps = psum.tile([128, J], F32, tag="ps")
    for q in range(J // 512):
        nc.tensor.matmul(featTj_ps[:, q * 512:(q + 1) * 512], W_sb[:],
                         xTj_sb[:, q * 512:(q + 1) * 512], start=True, stop=True)
    featTj_sb = pool.tile([128, J], F32, tag="featTj")
    nc.vector.tensor_copy(featTj_sb[:], featTj_ps[:])
    dstat = []
    for t in range(NT):
        sj_ps = psum.tile([128, 8], F32, tag="ps")
        nc.tensor.matmul(sj_ps[:], featTj_sb[:, t * 128:(t + 1) * 128],
                         attb_sb[:], start=True, stop=True)
        s = spool.tile([128, 8], F32, tag=f"stj_t{t}")
        nc.vector.tensor_copy(s[:], sj_ps[:])
        dstat.append(s)

    # per-tile exp stats on the j side (f32, used as per-partition scalars)
    e2c, rpc = [], []
    for t in range(NT):
        e2 = dpool.tile([128, 4], F32, tag=f"e2_{t}")
        nc.scalar.activation(e2[:], dstat[t][:, 4:8], AF.Exp)
        e2c.append(e2)
        rp = dpool.tile([128, 4], F32, tag=f"rp_{t}")
        nc.scalar.activation(rp[:], dstat[t][:, 4:8], AF.Exp,
                             scale=-(1.0 - NEG_SLOPE))
        rpc.append(rp)

    def bcast_row(row_ap, tag):
        """Replicate a [1, N] bf16 row to 128 partitions via doubling DMAs."""
        rep = rpool.tile([128, N], BF16, tag=tag)
        nc.sync.dma_start(rep[0:1, :], row_ap)
        p = 1
        while p < 128:
            nc.sync.dma_start(rep[p:2 * p, :], rep[0:p, :])
            p *= 2
        return rep

    # ---- lattice + output matmul, per head ----
    for h in range(H):
        # broadcast row h to 128 partitions (bf16)
        rep_src = bcast_row(asrc_bf[h:h + 1, :], "rep_src")
        rep_G = bcast_row(G1T[h:h + 1, :], "rep_G")
        rep_E1 = bcast_row(E1T[h:h + 1, :], "rep_E1")

        zt, dacc = [], []
        for t in range(NT):
            dcol = dpool.tile([128, 1], F32, tag=f"dacc{t}_{h % 2}")
            z = zpool.tile([128, N], BF16, tag="Z")
            if t < NT_ACT:
                # ACT pipe: adj tile holds BIG*(adj-1)
                u = upool.tile([128, N], BF16, tag="U")
                nc.vector.tensor_tensor(u[:], adj_sb[t][:], rep_src[:], OP.add)
                l = lpool.tile([128, N], F32, tag="L")
                nc.scalar.activation(l[:], u[:], AF.Prelu,
                                     bias=dstat[t][:, 4 + h:5 + h],
                                     alpha=NEG_SLOPE)
                nc.scalar.activation(z[:], l[:], AF.Exp, accum_out=dcol[:])
            else:
                # DVE pipe: adj tile holds 0/1
                k2 = upool.tile([128, N], BF16, tag="K2")
                nc.vector.tensor_scalar(k2[:], rep_G[:], rpc[t][:, h:h + 1], 1.0,
                                        OP.mult, OP.max)
                y = lpool.tile([128, N], BF16, tag="Y")
                eng = nc.gpsimd if GPS_Y else nc.vector
                eng.tensor_tensor(y[:], k2[:], rep_E1[:], OP.mult)
                zs = upool.tile([128, N], BF16, tag="ZS")
                nc.vector.tensor_tensor(zs[:], y[:], adj_sb[t][:], OP.mult)
                nc.vector.tensor_scalar(z[:], zs[:], 1.0, None, OP.mult,
                                        OP.add, accum_out=dcol[:])
            zt.append(z)
            dacc.append(dcol)

        # weights w[j, c] = feat/D (times E2 for DVE-pipe tiles)
        wts = []
        for t in range(NT):
            drec = dpool.tile([128, 1], F32, tag=f"drec{t}_{h % 2}")
            if t < NT_ACT:
                nc.vector.reciprocal(drec[:], dacc[t][:])
            else:
                dfull = dpool.tile([128, 1], F32, tag=f"dfull{t}_{h % 2}")
                nc.vector.tensor_scalar(dfull[:], dacc[t][:], e2c[t][:, h:h + 1],
                                        None, OP.mult)
                nc.vector.reciprocal(drec[:], dfull[:])
            w = dpool.tile([128, C], BF16, tag=f"w{t}_{h % 2}")
            if t < NT_ACT:
                nc.vector.tensor_scalar(w[:], fj_sb[t][:, h * C:(h + 1) * C],
                                        drec[:], None, OP.mult)
            else:
                nc.vector.tensor_scalar(w[:], fj_sb[t][:, h * C:(h + 1) * C],
                                        drec[:], e2c[t][:, h:h + 1],
                                        OP.mult, OP.mult)
            wts.append(w)

        # output matmul: psum [C, N] accumulated over j-tiles
        o_ps = psum.tile([C, N], F32, tag="ps")
        for t in range(NT):
            for q in range(N // 512):
                nc.tensor.matmul(o_ps[:, q * 512:(q + 1) * 512], wts[t][:],
                                 zt[t][:, q * 512:(q + 1) * 512],
                                 start=(t == 0), stop=(t == NT - 1))
        o_sb = opool.tile([C, N], F32, tag="osb")
        nc.vector.tensor_copy(o_sb[:], o_ps[:])
        nc.sync.dma_start(opart[h, :, :], o_sb[:])


_PROGRAM_CACHE = {}


def _build_program():
    if "nc" in _PROGRAM_CACHE:
        return _PROGRAM_CACHE["nc"]
    nc = bacc.Bacc("TRN2", target_bir_lowering=False, debug=False,
                   num_devices=N_CORES)
    adjm = nc.dram_tensor("adjm", [J, N], BF16, kind="ExternalInput").ap()
    xT = nc.dram_tensor("xT", [F_IN, N], F32, kind="ExternalInput").ap()
    xTj = nc.dram_tensor("xTj", [F_IN, J], F32, kind="ExternalInput").ap()
    Wt = nc.dram_tensor("Wt", [F_IN, HC], F32, kind="ExternalInput").ap()
    attb = nc.dram_tensor("attb", [HC, 8], F32, kind="ExternalInput").ap()
    opart = nc.dram_tensor("opart", [H, C, N], F32, kind="ExternalOutput").ap()
    with tile.TileContext(nc) as tc:
        _gat_kernel(tc, [opart], [adjm, xT, xTj, Wt, attb])
    nc.compile()
    _PROGRAM_CACHE["nc"] = nc
    return nc


def _make_in_maps(x, adj, W, att_src, att_dst):
    idx = np.arange(N)
    adj_sl = np.array(adj, dtype=np.float32, copy=True)
    adj_sl[:, idx, idx] = 1.0
    attb = np.zeros((HC, 8), dtype=np.float32)
    for h in range(H):
        attb[h * C:(h + 1) * C, h] = att_src[h]
        attb[h * C:(h + 1) * C, 4 + h] = att_dst[h]
    in_maps = []
    for k in range(N_CORES):
        b, half = k // 2, k % 2
        j0 = half * J
        adjT = adj_sl[b, :, j0:j0 + J].T.astype(np.float32)   # [J, N]
        adjm = np.empty((J, N), dtype=ml_dtypes.bfloat16)
        na = NT_ACT * 128
        adjm[:na] = (BIG * (adjT[:na] - 1.0)).astype(ml_dtypes.bfloat16)
        adjm[na:] = adjT[na:].astype(ml_dtypes.bfloat16)
        xTb = np.ascontiguousarray(x[b].T, dtype=np.float32)   # [F, N]
        xTjb = np.ascontiguousarray(x[b, j0:j0 + J].T, dtype=np.float32)
        in_maps.append({
            "adjm": adjm,
            "xT": xTb,
            "xTj": xTjb,
            "Wt": np.ascontiguousarray(W, dtype=np.float32),
            "attb": attb,
        })
    return in_maps


def _gather(results, bias):
    o = np.zeros((B, N, HC), dtype=np.float32)
    for k, res in enumerate(results):
        b = k // 2
        part = res["opart"]                      # [H, C, N]
        o[b] += part.transpose(2, 0, 1).reshape(N, HC)
    return o + bias.astype(np.float32)[None, None, :]


def kernel(x, adj, W, att_src, att_dst, bias, _run_kwargs=None):
    nc = _build_program()
    in_maps = _make_in_maps(np.asarray(x), np.asarray(adj), np.asarray(W),
                            np.asarray(att_src), np.asarray(att_dst))
    kw = dict(_run_kwargs or {})
    res = run_bass_kernel_spmd(nc, in_maps, core_ids=list(range(N_CORES)), **kw)
    out = _gather(res.results, np.asarray(bias))
    kernel.last_results = res
    return out


# revision 10
# speedup vs baseline: 1.1547x; 1.1547x over previous
"""DenseGATConv Trainium2 kernel (8 NeuronCores).

Math (per batch b, head h, with e = exp o leaky_relu_{0.2}):
    feat = x @ W                          # [N, H*C]
    a_src[i,h] = <feat[i, h,:], att_src[h]> ; a_dst[j,h] likewise
    T[i,j,h]   = a_src[i,h] + a_dst[j,h]
    alpha      = adj_sl[i,j] * e(T)       # adj_sl = adj with unit diagonal
    D[j,h]     = sum_i alpha[i,j,h]       # normalization over axis i
    o[i,h,:]   = sum_j alpha[i,j,h] * feat[j,h,:] / D[j,h]  (+ bias)

Sharding: 8 cores = (b in 0..3) x (j-half in 0..1); host sums the two
j-half partial outputs per batch.

Device layout: [j partitions, i free] (adj arrives pre-transposed, bf16).
Per (h, j-tile) one of three pipes produces Z[j,i] (the TensorE stream,
= alpha for P2/P1, = alpha for P3 with E2 folded in) and Dacc[j] = D:

  P2 "PE-U"  (adj tile = BIG*(adj-1)):  U = ones x a_src_row + I @ adjm on
     TensorE into PSUM; L = Prelu(U + a_dst[j]); Z,D = Exp(L) w/ accum.
  P1 "DVE-U": U = adjm + a_src_rep on VectorE; then as P2.
  P3 "DVE"   (adj tile = 0/1), with e(T) = E2[j]*E1[i]*max(1, G'[i]r'[j]):
     K2 = (G'_rep * r'[j]) max 1     (tensor_scalar, bf16 4x)
     Y  = K2 * E1_rep                (tensor_tensor, bf16 2x)
     Z  = (Y * E2[j]) * adj01, Dacc = row sums  (scalar_tensor_tensor+accum)

Output matmul per h: lhsT = w[j,c] = feat[j,c]/D[j], rhs = Z, accumulated
over j-tiles into PSUM [C, N]; evicted via ScalarE; summed across j-half
cores on the host.
"""

import numpy as np
import ml_dtypes

from contextlib import ExitStack

import concourse.bass as bass
import concourse.tile as tile
from concourse import bacc, mybir
from concourse._compat import with_exitstack
from concourse.bass_utils import run_bass_kernel_spmd

F32 = mybir.dt.float32
BF16 = mybir.dt.bfloat16
AF = mybir.ActivationFunctionType
OP = mybir.AluOpType

B, N, F_IN, H, C = 4, 2048, 128, 4, 32
HC = H * C
J = N // 2          # destination columns per core
NT = J // 128       # j-tiles per core (8)
NEG_SLOPE = 0.2
BIG = 128.0
N_CORES = 8

# --- tuning: pipe per j-tile (same for every head) --------------------------
# 'P2' = PE-built U, 'P1' = DVE-built U (both BIG-form adj, ScalarE exp path)
# 'P3' = factored DVE pipe (0/1-form adj)
PIPES = ["P2", "P2", "P2", "P1", "P3", "P3", "P3", "P3"]
NT_BIG = sum(1 for p in PIPES if p != "P3")   # host: tiles < NT_BIG in BIG form


@with_exitstack
def _gat_kernel(ctx: ExitStack, tc: tile.TileContext, outs, ins):
    nc = tc.nc
    adjm, xT, xTj, Wt, attb, ident = ins
    opart = outs[0]                    # dram [H, C, N] f32

    pool = ctx.enter_context(tc.tile_pool(name="work", bufs=1))
    zpool = ctx.enter_context(tc.tile_pool(name="z", bufs=10))
    upool = ctx.enter_context(tc.tile_pool(name="u", bufs=2))
    lpool = ctx.enter_context(tc.tile_pool(name="l", bufs=3))
    kpool = ctx.enter_context(tc.tile_pool(name="k", bufs=2))
    rpool = ctx.enter_context(tc.tile_pool(name="reps", bufs=2))
    spool = ctx.enter_context(tc.tile_pool(name="small", bufs=1))
    dpool = ctx.enter_context(tc.tile_pool(name="dcols", bufs=1))
    psum = ctx.enter_context(tc.tile_pool(name="ps", bufs=1, space="PSUM"))
    psu = ctx.enter_context(tc.tile_pool(name="psu", bufs=2, space="PSUM"))

    # ---- small inputs first (stage 1 gates everything) ----
    xT_sb = pool.tile([128, N], F32, tag="xT")
    nc.sync.dma_start(xT_sb[:], xT[:, :])
    xTj_sb = pool.tile([128, J], F32, tag="xTj")
    nc.sync.dma_start(xTj_sb[:], xTj[:, :])
    W_sb = spool.tile([128, HC], F32, tag="W")
    nc.sync.dma_start(W_sb[:], Wt[:, :])
    attb_sb = spool.tile([128, 8], F32, tag="attb")
    nc.sync.dma_start(attb_sb[:], attb[:, :])
    id_sb = spool.tile([128, 128], BF16, tag="ident")
    nc.sync.dma_start(id_sb[:], ident[:, :])
    ones1 = spool.tile([1, 128], BF16, tag="ones1")
    nc.vector.memset(ones1[:], 1.0)

    adj_sb = []                        # [128, 2048] bf16 per j-tile
    for t in range(NT):
        a = pool.tile([128, N], BF16, tag=f"adjm{t}")
        nc.sync.dma_start(a[:], adjm[t * 128:(t + 1) * 128, :])
        adj_sb.append(a)

    # ---- projections ----
    featT_ps = psum.tile([128, N], F32, tag="ps")
    for q in range(N // 512):
        nc.tensor.matmul(featT_ps[:, q * 512:(q + 1) * 512], W_sb[:],
                         xT_sb[:, q * 512:(q + 1) * 512], start=True, stop=True)
    featT_sb = pool.tile([128, N], F32, tag="featT")
    nc.scalar.copy(featT_sb[:], featT_ps[:])

    featTj_ps = psu.tile([128, J], F32, tag="psu")
    for q in range(J // 512):
        nc.tensor.matmul(featTj_ps[:, q * 512:(q + 1) * 512], W_sb[:],
                         xTj_sb[:, q * 512:(q + 1) * 512], start=True, stop=True)
    featTj_sb = pool.tile([128, J], F32, tag="featTj")
    nc.scalar.copy(featTj_sb[:], featTj_ps[:])

    # feat_j[j, hc] per j-tile (weights source)
    fj_sb = []
    for t in range(NT):
        fj_ps = psu.tile([128, HC], F32, tag="psu")
        nc.tensor.matmul(fj_ps[:], xTj_sb[:, t * 128:(t + 1) * 128], W_sb[:],
                         start=True, stop=True)
        f = pool.tile([128, HC], F32, tag=f"fj{t}")
        nc.vector.tensor_copy(f[:], fj_ps[:])
        fj_sb.append(f)

    # ---- attention stats ----
    stats_ps = psum.tile([8, N], F32, tag="ps")
    for q in range(N // 512):
        nc.tensor.matmul(stats_ps[:, q * 512:(q + 1) * 512], attb_sb[:],
                         featT_sb[:, q * 512:(q + 1) * 512], start=True, stop=True)
    asrc4 = spool.tile([4, N], F32, tag="asrc4")
    nc.vector.tensor_copy(asrc4[:], stats_ps[0:4, :])
    asrc_bf = spool.tile([4, N], BF16, tag="asrc_bf")
    nc.vector.tensor_copy(asrc_bf[:], asrc4[:])
    G1T = spool.tile([4, N], BF16, tag="G1T")
    nc.scalar.activation(G1T[:], asrc4[:], AF.Exp, scale=-(1.0 - NEG_SLOPE))
    E1T = spool.tile([4, N], BF16, tag="E1T")
    nc.scalar.activation(E1T[:], asrc4[:], AF.Exp)

    # j-side stats per tile: [j, s] via lhsT = featTj chunk (contract hc)
    dstat, e2c, rpc = [], [], []
    for t in range(NT):
        sj_ps = psu.tile([128, 8], F32, tag="psu")
        nc.tensor.matmul(sj_ps[:], featTj_sb[:, t * 128:(t + 1) * 128],
                         attb_sb[:], start=True, stop=True)
        s = spool.tile([128, 8], F32, tag=f"stj_t{t}")
        nc.vector.tensor_copy(s[:], sj_ps[:])
        dstat.append(s)
        e2 = dpool.tile([128, 4], F32, tag=f"e2_{t}")
        nc.scalar.activation(e2[:], s[:, 4:8], AF.Exp)
        e2c.append(e2)
        rp = dpool.tile([128, 4], F32, tag=f"rp_{t}")
        nc.scalar.activation(rp[:], s[:, 4:8], AF.Exp, scale=-(1.0 - NEG_SLOPE))
        rpc.append(rp)

    def bcast_row(row_ap, tag):
        """Replicate a [1, N] bf16 row to 128 partitions via doubling DMAs."""
        rep = rpool.tile([128, N], BF16, tag=tag)
        nc.sync.dma_start(rep[0:1, :], row_ap)
        p = 1
        while p < 128:
            nc.sync.dma_start(rep[p:2 * p, :], rep[0:p, :])
            p *= 2
        return rep

    NEED_SRC = any(p == "P1" for p in PIPES)
    NEED_GE = any(p == "P3" for p in PIPES)

    # ---- lattice + output matmul, per head ----
    for h in range(H):
        rep_src = bcast_row(asrc_bf[h:h + 1, :], "rep_src") if NEED_SRC else None
        rep_G = bcast_row(G1T[h:h + 1, :], "rep_G") if NEED_GE else None
        rep_E1 = bcast_row(E1T[h:h + 1, :], "rep_E1") if NEED_GE else None
        # head row staged at base partition 0 (matmul rhs requirement)
        asrc_row = rpool.tile([1, N], BF16, tag="asrc_row")
        nc.sync.dma_start(asrc_row[:], asrc_bf[h:h + 1, :])

        zt, dacc = [], []
        for t in range(NT):
            dcol = dpool.tile([128, 1], F32, tag=f"dacc{t}_{h % 2}")
            z = zpool.tile([128, N], BF16, tag="Z")
            pipe = PIPES[t]
            if pipe == "P2":
                # U built on TensorE into PSUM, in two i-halves
                for q in range(2):
                    NH = N // 2
                    ups = psu.tile([128, NH], F32, tag="psu")
                    for r in range(NH // 512):
                        sl = slice(q * NH + r * 512, q * NH + (r + 1) * 512)
                        psl = slice(r * 512, (r + 1) * 512)
                        nc.tensor.matmul(ups[:, psl], ones1[:],
                                         asrc_row[0:1, sl],
                                         start=True, stop=False)
                        nc.tensor.matmul(ups[:, psl], id_sb[:],
                                         adj_sb[t][:, sl],
                                         start=False, stop=True)
                    l = lpool.tile([128, NH], BF16, tag="Lh")
                    nc.scalar.activation(l[:], ups[:], AF.Prelu,
                                         bias=dstat[t][:, 4 + h:5 + h],
                                         alpha=NEG_SLOPE)
                    dq = dpool.tile([128, 1], F32, tag=f"dq{q}_{t % 2}_{h % 2}")
                    nc.scalar.activation(z[:, q * NH:(q + 1) * NH], l[:],
                                         AF.Exp, accum_out=dq[:])
                    if q == 0:
                        dq0 = dq
                    else:
                        nc.vector.tensor_tensor(dcol[:], dq0[:], dq[:], OP.add)
            elif pipe == "P1":
                u = upool.tile([128, N], BF16, tag="U")
                nc.vector.tensor_tensor(u[:], adj_sb[t][:], rep_src[:], OP.add)
                l = lpool.tile([128, N], BF16, tag="L")
                nc.scalar.activation(l[:], u[:], AF.Prelu,
                                     bias=dstat[t][:, 4 + h:5 + h],
                                     alpha=NEG_SLOPE)
                nc.scalar.activation(z[:], l[:], AF.Exp, accum_out=dcol[:])
            else:  # P3
                k2 = kpool.tile([128, N], BF16, tag="K2")
                nc.vector.tensor_scalar(k2[:], rep_G[:], rpc[t][:, h:h + 1], 1.0,
                                        OP.mult, OP.max)
                y = kpool.tile([128, N], BF16, tag="Y")
                nc.vector.tensor_tensor(y[:], k2[:], rep_E1[:], OP.mult)
                nc.vector.scalar_tensor_tensor(z[:], y[:], e2c[t][:, h:h + 1],
                                               adj_sb[t][:], OP.mult, OP.mult,
                                               accum_out=dcol[:])
            zt.append(z)
            dacc.append(dcol)

        # weights w[j, c] = feat/D
        wts = []
        for t in range(NT):
            drec = dpool.tile([128, 1], F32, tag=f"drec{t}_{h % 2}")
            nc.vector.reciprocal(drec[:], dacc[t][:])
            w = dpool.tile([128, C], BF16, tag=f"w{t}_{h % 2}")
            nc.vector.tensor_scalar(w[:], fj_sb[t][:, h * C:(h + 1) * C],
                                    drec[:], None, OP.mult)
            wts.append(w)

        # output matmul: psum [C, N] accumulated over j-tiles
        o_ps = psum.tile([C, N], F32, tag="ps")
        for t in range(NT):
            for q in range(N // 512):
                nc.tensor.matmul(o_ps[:, q * 512:(q + 1) * 512], wts[t][:],
                                 zt[t][:, q * 512:(q + 1) * 512],
                                 start=(t == 0), stop=(t == NT - 1))
        o_sb = spool.tile([C, N], F32, tag="osb")
        nc.scalar.copy(o_sb[:], o_ps[:])
        nc.sync.dma_start(opart[h, :, :], o_sb[:])


_PROGRAM_CACHE = {}


def _build_program():
    if "nc" in _PROGRAM_CACHE:
        return _PROGRAM_CACHE["nc"]
    nc = bacc.Bacc("TRN2", target_bir_lowering=False, debug=False,
                   num_devices=N_CORES)
    adjm = nc.dram_tensor("adjm", [J, N], BF16, kind="ExternalInput").ap()
    xT = nc.dram_tensor("xT", [F_IN, N], F32, kind="ExternalInput").ap()
    xTj = nc.dram_tensor("xTj", [F_IN, J], F32, kind="ExternalInput").ap()
    Wt = nc.dram_tensor("Wt", [F_IN, HC], F32, kind="ExternalInput").ap()
    attb = nc.dram_tensor("attb", [HC, 8], F32, kind="ExternalInput").ap()
    ident = nc.dram_tensor("ident", [128, 128], BF16, kind="ExternalInput").ap()
    opart = nc.dram_tensor("opart", [H, C, N], F32, kind="ExternalOutput").ap()
    with tile.TileContext(nc) as tc:
        _gat_kernel(tc, [opart], [adjm, xT, xTj, Wt, attb, ident])
    nc.compile()
    _PROGRAM_CACHE["nc"] = nc
    return nc


def _make_in_maps(x, adj, W, att_src, att_dst):
    idx = np.arange(N)
    adj_sl = np.array(adj, dtype=np.float32, copy=True)
    adj_sl[:, idx, idx] = 1.0
    attb = np.zeros((HC, 8), dtype=np.float32)
    for h in range(H):
        attb[h * C:(h + 1) * C, h] = att_src[h]
        attb[h * C:(h + 1) * C, 4 + h] = att_dst[h]
    ident = np.eye(128, dtype=ml_dtypes.bfloat16)
    in_maps = []
    for k in range(N_CORES):
        b, half = k // 2, k % 2
        j0 = half * J
        adjT = adj_sl[b, :, j0:j0 + J].T.astype(np.float32)   # [J, N]
        adjm = np.empty((J, N), dtype=ml_dtypes.bfloat16)
        nb = NT_BIG * 128
        adjm[:nb] = (BIG * (adjT[:nb] - 1.0)).astype(ml_dtypes.bfloat16)
        adjm[nb:] = adjT[nb:].astype(ml_dtypes.bfloat16)
        in_maps.append({
            "adjm": adjm,
            "xT": np.ascontiguousarray(x[b].T, dtype=np.float32),
            "xTj": np.ascontiguousarray(x[b, j0:j0 + J].T, dtype=np.float32),
            "Wt": np.ascontiguousarray(W, dtype=np.float32),
            "attb": attb,
            "ident": ident,
        })
    return in_maps


def _gather(results, bias):
    o = np.zeros((B, N, HC), dtype=np.float32)
    for k, res in enumerate(results):
        b = k // 2
        part = res["opart"]                      # [H, C, N]
        o[b] += part.transpose(2, 0, 1).reshape(N, HC)
    return o + bias.astype(np.float32)[None, None, :]


def kernel(x, adj, W, att_src, att_dst, bias, _run_kwargs=None):
    nc = _build_program()
    in_maps = _make_in_maps(np.asarray(x), np.asarray(adj), np.asarray(W),
                            np.asarray(att_src), np.asarray(att_dst))
    kw = dict(_run_kwargs or {})
    res = run_bass_kernel_spmd(nc, in_maps, core_ids=list(range(N_CORES)), **kw)
    out = _gather(res.results, np.asarray(bias))
    kernel.last_results = res
    return out


# revision 13
# speedup vs baseline: 1.2724x; 1.1019x over previous
"""DenseGATConv Trainium2 kernel (8 NeuronCores).

Math (per batch b, head h, with e = exp o leaky_relu_{0.2}):
    feat = x @ W                          # [N, H*C]
    a_src[i,h] = <feat[i, h,:], att_src[h]> ; a_dst[j,h] likewise
    T[i,j,h]   = a_src[i,h] + a_dst[j,h]
    alpha      = adj_sl[i,j] * e(T)       # adj_sl = adj with unit diagonal
    D[j,h]     = sum_i alpha[i,j,h]       # normalization over axis i
    o[i,h,:]   = sum_j alpha[i,j,h] * feat[j,h,:] / D[j,h]  (+ bias)

Sharding: 8 cores = (b in 0..3) x (j-half in 0..1); host sums the two
j-half partial outputs per batch.

Device layout: [j partitions, i free] (adj arrives pre-transposed, bf16).
Per (h, j-tile) one of three pipes produces Z[j,i] (the TensorE stream,
= alpha for P2/P1, = alpha for P3 with E2 folded in) and Dacc[j] = D:

  P2 "PE-U"  (adj tile = BIG*(adj-1)):  U = ones x a_src_row + I @ adjm on
     TensorE into PSUM; L = Prelu(U + a_dst[j]); Z,D = Exp(L) w/ accum.
  P1 "DVE-U": U = adjm + a_src_rep on VectorE; then as P2.
  P3 "DVE"   (adj tile = 0/1), with e(T) = E2[j]*E1[i]*max(1, G'[i]r'[j]):
     K2 = (G'_rep * r'[j]) max 1     (tensor_scalar, bf16 4x)
     Y  = K2 * E1_rep                (tensor_tensor, bf16 2x)
     Z  = (Y * E2[j]) * adj01, Dacc = row sums  (scalar_tensor_tensor+accum)

Output matmul per h: lhsT = w[j,c] = feat[j,c]/D[j], rhs = Z, accumulated
over j-tiles into PSUM [C, N]; evicted via ScalarE; summed across j-half
cores on the host.
"""

import numpy as np
import ml_dtypes

from contextlib import ExitStack

import concourse.bass as bass
import concourse.tile as tile
from concourse import bacc, mybir
from concourse._compat import with_exitstack
from concourse.bass_utils import run_bass_kernel_spmd

F32 = mybir.dt.float32
BF16 = mybir.dt.bfloat16
AF = mybir.ActivationFunctionType
OP = mybir.AluOpType

B, N, F_IN, H, C = 4, 2048, 128, 4, 32
HC = H * C
J = N // 2          # destination columns per core
NT = J // 128       # j-tiles per core (8)
NEG_SLOPE = 0.2
BIG = 128.0
N_CORES = 8

# --- tuning: pipe per j-tile (same for every head) --------------------------
# 'P2' = PE-built U, 'P1' = DVE-built U (both BIG-form adj, ScalarE exp path)
# 'P3' = factored DVE pipe (0/1-form adj)
PIPES = ["P2", "P1", "P1", "P1", "P3", "P3", "P3", "P3"]
NT_BIG = sum(1 for p in PIPES if p != "P3")   # host: tiles < NT_BIG in BIG form


@with_exitstack
def _gat_kernel(ctx: ExitStack, tc: tile.TileContext, outs, ins):
    nc = tc.nc
    adjm, xT, xTj, Wt, attb, ident = ins
    opart = outs[0]                    # dram [H, C, N] f32

    pool = ctx.enter_context(tc.tile_pool(name="work", bufs=1))
    zpool = ctx.enter_context(tc.tile_pool(name="z", bufs=10))
    upool = ctx.enter_context(tc.tile_pool(name="u", bufs=2))
    lpool = ctx.enter_context(tc.tile_pool(name="l", bufs=3))
    kpool = ctx.enter_context(tc.tile_pool(name="k", bufs=2))
    rpool = ctx.enter_context(tc.tile_pool(name="reps", bufs=2))
    spool = ctx.enter_context(tc.tile_pool(name="small", bufs=1))
    dpool = ctx.enter_context(tc.tile_pool(name="dcols", bufs=1))
    psum = ctx.enter_context(tc.tile_pool(name="ps", bufs=1, space="PSUM"))
    psu = ctx.enter_context(tc.tile_pool(name="psu", bufs=2, space="PSUM"))

    # ---- small inputs first (stage 1 gates everything) ----
    xT_sb = pool.tile([128, N], F32, tag="xT")
    nc.sync.dma_start(xT_sb[:], xT[:, :])
    xTj_sb = pool.tile([128, J], F32, tag="xTj")
    nc.sync.dma_start(xTj_sb[:], xTj[:, :])
    W_sb = spool.tile([128, HC], F32, tag="W")
    nc.sync.dma_start(W_sb[:], Wt[:, :])
    attb_sb = spool.tile([128, 8], F32, tag="attb")
    nc.sync.dma_start(attb_sb[:], attb[:, :])
    id_sb = spool.tile([128, 128], BF16, tag="ident")
    nc.sync.dma_start(id_sb[:], ident[:, :])
    ones1 = spool.tile([1, 128], BF16, tag="ones1")
    nc.vector.memset(ones1[:], 1.0)

    adj_sb = []                        # [128, 2048] bf16 per j-tile
    for t in range(NT):
        a = pool.tile([128, N], BF16, tag=f"adjm{t}")
        nc.sync.dma_start(a[:], adjm[t * 128:(t + 1) * 128, :])
        adj_sb.append(a)

    # ---- projections ----
    featT_ps = psum.tile([128, N], F32, tag="ps")
    for q in range(N // 512):
        nc.tensor.matmul(featT_ps[:, q * 512:(q + 1) * 512], W_sb[:],
                         xT_sb[:, q * 512:(q + 1) * 512], start=True, stop=True)
    featT_sb = pool.tile([128, N], F32, tag="featT")
    nc.scalar.copy(featT_sb[:], featT_ps[:])

    featTj_ps = psu.tile([128, J], F32, tag="psu")
    for q in range(J // 512):
        nc.tensor.matmul(featTj_ps[:, q * 512:(q + 1) * 512], W_sb[:],
                         xTj_sb[:, q * 512:(q + 1) * 512], start=True, stop=True)
    featTj_sb = pool.tile([128, J], F32, tag="featTj")
    nc.scalar.copy(featTj_sb[:], featTj_ps[:])

    # feat_j[j, hc] per j-tile (weights source)
    fj_sb = []
    for t in range(NT):
        fj_ps = psu.tile([128, HC], F32, tag="psu")
        nc.tensor.matmul(fj_ps[:], xTj_sb[:, t * 128:(t + 1) * 128], W_sb[:],
                         start=True, stop=True)
        f = pool.tile([128, HC], F32, tag=f"fj{t}")
        nc.vector.tensor_copy(f[:], fj_ps[:])
        fj_sb.append(f)

    # ---- attention stats ----
    stats_ps = psum.tile([8, N], F32, tag="ps")
    for q in range(N // 512):
        nc.tensor.matmul(stats_ps[:, q * 512:(q + 1) * 512], attb_sb[:],
                         featT_sb[:, q * 512:(q + 1) * 512], start=True, stop=True)
    asrc4 = spool.tile([4, N], F32, tag="asrc4")
    nc.vector.tensor_copy(asrc4[:], stats_ps[0:4, :])
    asrc_bf = spool.tile([4, N], BF16, tag="asrc_bf")
    nc.vector.tensor_copy(asrc_bf[:], asrc4[:])
    G1T = spool.tile([4, N], BF16, tag="G1T")
    nc.scalar.activation(G1T[:], asrc4[:], AF.Exp, scale=-(1.0 - NEG_SLOPE))
    E1T = spool.tile([4, N], BF16, tag="E1T")
    nc.scalar.activation(E1T[:], asrc4[:], AF.Exp)

    # j-side stats per tile: [j, s] via lhsT = featTj chunk (contract hc)
    dstat, e2c, rpc = [], [], []
    for t in range(NT):
        sj_ps = psu.tile([128, 8], F32, tag="psu")
        nc.tensor.matmul(sj_ps[:], featTj_sb[:, t * 128:(t + 1) * 128],
                         attb_sb[:], start=True, stop=True)
        s = spool.tile([128, 8], F32, tag=f"stj_t{t}")
        nc.vector.tensor_copy(s[:], sj_ps[:])
        dstat.append(s)
        e2 = dpool.tile([128, 4], F32, tag=f"e2_{t}")
        nc.scalar.activation(e2[:], s[:, 4:8], AF.Exp)
        e2c.append(e2)
        rp = dpool.tile([128, 4], F32, tag=f"rp_{t}")
        nc.scalar.activation(rp[:], s[:, 4:8], AF.Exp, scale=-(1.0 - NEG_SLOPE))
        rpc.append(rp)

    # ---- lattice + output matmul, per head ----
    for h in range(H):
        # one combined [128, 3N] broadcast per head: a_src | G' | E1 rows
        crep = rpool.tile([128, 3 * N], BF16, tag="crep")
        nc.sync.dma_start(crep[0:1, 0:N], asrc_bf[h:h + 1, :])
        nc.sync.dma_start(crep[0:1, N:2 * N], G1T[h:h + 1, :])
        nc.sync.dma_start(crep[0:1, 2 * N:3 * N], E1T[h:h + 1, :])
        p = 1
        while p < 128:
            nc.sync.dma_start(crep[p:2 * p, :], crep[0:p, :])
            p *= 2
        rep_src = crep[:, 0:N]
        rep_G = crep[:, N:2 * N]
        rep_E1 = crep[:, 2 * N:3 * N]
        # head row staged at base partition 0 (matmul rhs requirement)
        asrc_row = rpool.tile([1, N], BF16, tag="asrc_row")
        nc.sync.dma_start(asrc_row[:], asrc_bf[h:h + 1, :])

        o_ps = psum.tile([C, N], F32, tag="ps")
        for t in range(NT):
            dcol = dpool.tile([128, 1], F32, tag=f"dacc{t}_{h % 2}")
            z = zpool.tile([128, N], BF16, tag="Z")
            pipe = PIPES[t]
            if pipe == "P2":
                # U built on TensorE into PSUM, in two i-halves
                for q in range(2):
                    NH = N // 2
                    ups = psu.tile([128, NH], F32, tag="psu")
                    for r in range(NH // 512):
                        sl = slice(q * NH + r * 512, q * NH + (r + 1) * 512)
                        psl = slice(r * 512, (r + 1) * 512)
                        nc.tensor.matmul(ups[:, psl], ones1[:],
                                         asrc_row[0:1, sl],
                                         start=True, stop=False)
                        nc.tensor.matmul(ups[:, psl], id_sb[:],
                                         adj_sb[t][:, sl],
                                         start=False, stop=True)
                    l = lpool.tile([128, NH], BF16, tag="Lh")
                    nc.scalar.activation(l[:], ups[:], AF.Prelu,
                                         bias=dstat[t][:, 4 + h:5 + h],
                                         alpha=NEG_SLOPE)
                    dq = dpool.tile([128, 1], F32, tag=f"dq{q}_{t % 2}_{h % 2}")
                    nc.scalar.activation(z[:, q * NH:(q + 1) * NH], l[:],
                                         AF.Exp, accum_out=dq[:])
                    if q == 0:
                        dq0 = dq
                    else:
                        nc.vector.tensor_tensor(dcol[:], dq0[:], dq[:], OP.add)
            elif pipe == "P1":
                u = upool.tile([128, N], BF16, tag="U")
                nc.vector.tensor_tensor(u[:], adj_sb[t][:], rep_src[:], OP.add)
                l = lpool.tile([128, N], BF16, tag="L")
                nc.scalar.activation(l[:], u[:], AF.Prelu,
                                     bias=dstat[t][:, 4 + h:5 + h],
                                     alpha=NEG_SLOPE)
                nc.scalar.activation(z[:], l[:], AF.Exp, accum_out=dcol[:])
            else:  # P3
                k2 = kpool.tile([128, N], BF16, tag="K2")
                nc.vector.tensor_scalar(k2[:], rep_G[:], rpc[t][:, h:h + 1], 1.0,
                                        OP.mult, OP.max)
                y = kpool.tile([128, N], BF16, tag="Y")
                nc.vector.tensor_tensor(y[:], k2[:], rep_E1[:], OP.mult)
                nc.vector.scalar_tensor_tensor(z[:], y[:], e2c[t][:, h:h + 1],
                                               adj_sb[t][:], OP.mult, OP.mult,
                                               accum_out=dcol[:])

            # weights + output matmul for this tile (keeps TensorE streaming)
            drec = dpool.tile([128, 1], F32, tag=f"drec{t}_{h % 2}")
            nc.vector.reciprocal(drec[:], dcol[:])
            w = dpool.tile([128, C], BF16, tag=f"w{t}_{h % 2}")
            nc.vector.tensor_scalar(w[:], fj_sb[t][:, h * C:(h + 1) * C],
                                    drec[:], None, OP.mult)
            for q in range(N // 512):
                nc.tensor.matmul(o_ps[:, q * 512:(q + 1) * 512], w[:],
                                 z[:, q * 512:(q + 1) * 512],
                                 start=(t == 0), stop=(t == NT - 1))

        o_sb = spool.tile([C, N], F32, tag="osb")
        nc.scalar.copy(o_sb[:], o_ps[:])
        nc.sync.dma_start(opart[h, :, :], o_sb[:])


_PROGRAM_CACHE = {}


def _build_program():
    if "nc" in _PROGRAM_CACHE:
        return _PROGRAM_CACHE["nc"]
    nc = bacc.Bacc("TRN2", target_bir_lowering=False, debug=False,
                   num_devices=N_CORES)
    adjm = nc.dram_tensor("adjm", [J, N], BF16, kind="ExternalInput").ap()
    xT = nc.dram_tensor("xT", [F_IN, N], F32, kind="ExternalInput").ap()
    xTj = nc.dram_tensor("xTj", [F_IN, J], F32, kind="ExternalInput").ap()
    Wt = nc.dram_tensor("Wt", [F_IN, HC], F32, kind="ExternalInput").ap()
    attb = nc.dram_tensor("attb", [HC, 8], F32, kind="ExternalInput").ap()
    ident = nc.dram_tensor("ident", [128, 128], BF16, kind="ExternalInput").ap()
    opart = nc.dram_tensor("opart", [H, C, N], F32, kind="ExternalOutput").ap()
    with tile.TileContext(nc) as tc:
        _gat_kernel(tc, [opart], [adjm, xT, xTj, Wt, attb, ident])
    nc.compile()
    _PROGRAM_CACHE["nc"] = nc
    return nc


def _make_in_maps(x, adj, W, att_src, att_dst):
    idx = np.arange(N)
    adj_sl = np.array(adj, dtype=np.float32, copy=True)
    adj_sl[:, idx, idx] = 1.0
    attb = np.zeros((HC, 8), dtype=np.float32)
    for h in range(H):
        attb[h * C:(h + 1) * C, h] = att_src[h]
        attb[h * C:(h + 1) * C, 4 + h] = att_dst[h]
    ident = np.eye(128, dtype=ml_dtypes.bfloat16)
    in_maps = []
    for k in range(N_CORES):
        b, half = k // 2, k % 2
        j0 = half * J
        adjT = adj_sl[b, :, j0:j0 + J].T.astype(np.float32)   # [J, N]
        adjm = np.empty((J, N), dtype=ml_dtypes.bfloat16)
        nb = NT_BIG * 128
        adjm[:nb] = (BIG * (adjT[:nb] - 1.0)).astype(ml_dtypes.bfloat16)
        adjm[nb:] = adjT[nb:].astype(ml_dtypes.bfloat16)
        in_maps.append({
            "adjm": adjm,
            "xT": np.ascontiguousarray(x[b].T, dtype=np.float32),
            "xTj": np.ascontiguousarray(x[b, j0:j0 + J].T, dtype=np.float32),
            "Wt": np.ascontiguousarray(W, dtype=np.float32),
            "attb": attb,
            "ident": ident,
        })
    return in_maps


def _gather(results, bias):
    o = np.zeros((B, N, HC), dtype=np.float32)
    for k, res in enumerate(results):
        b = k // 2
        part = res["opart"]                      # [H, C, N]
        o[b] += part.transpose(2, 0, 1).reshape(N, HC)
    return o + bias.astype(np.float32)[None, None, :]


def kernel(x, adj, W, att_src, att_dst, bias, _run_kwargs=None):
    nc = _build_program()
    in_maps = _make_in_maps(np.asarray(x), np.asarray(adj), np.asarray(W),
                            np.asarray(att_src), np.asarray(att_dst))
    kw = dict(_run_kwargs or {})
    res = run_bass_kernel_spmd(nc, in_maps, core_ids=list(range(N_CORES)), **kw)
    out = _gather(res.results, np.asarray(bias))
    kernel.last_results = res
    return out
